# revision 1
# baseline (speedup 1.0000x reference)
"""Trainium2 Bass kernel for nn_BackProjNet (segment_reduce).

out[c, v] = (sum_r x[c, idx[v, r]] * w[v, r]) * SCALE + bias[v]

Strategy (8 NeuronCores, voxel-sharded):
  - Each core owns 8192 voxels (voxel v -> partition p = v//64, layer jj = v%64).
  - Positions (92160) are split into 3 "colors" of 30720 so row ids fit int16;
    the per-core gather table is a 256B-pitch f32 table [92160, 64] with the
    8-channel payload in the first 32B, rows grouped by color.
  - Per (voxel, color) refs are padded to K slots (weight 0 on pads); gathers
    run as InstDMAGatherAnt (32B elements, 256B stride) on 4 SWDGE queues,
    1024 indices per instruction.
  - DVE multiplies by the weight stream (broadcast over channels) and does a
    strided segment reduce; bias is added once at the end.
"""

import sys

import numpy as np

for _p in ("/opt/trn_rl_repo", "/root/.axon_site/_ro/trn_rl_repo"):
    if _p not in sys.path:
        sys.path.append(_p)

import concourse.bass as bass
import concourse.bacc as bacc
import concourse.mybir as mybir
import concourse.tile as tile
from concourse import ap_utils
from concourse._compat import exact_div
from concourse.bass import round_up_to_multiple
from concourse.bass_interp import get_hw_module

# geometry (must match reference.py)
CHANNEL = 8
NVX, NVY = 256, 256
VIEWS, EXTENT = 180, 2
NDETU = 512
SCALE = (2.0 * np.pi - 0.0) / (2.0 * VIEWS * EXTENT)

NCORES = 8
P = 128
V = VIEWS * NDETU          # 92160 sinogram positions
R = VIEWS * EXTENT         # 360 rays per voxel
NVOX = NVX * NVY           # 65536 voxels
VPC = NVOX // NCORES       # 8192 voxels per core
JPP = VPC // P             # 64 voxels per partition ("layers")
COLORS = 3
CSIZE = V // COLORS        # 30720 rows per color sub-table
PITCH = 64                 # f32 per table row (256B)
NPI = 1024                 # indices per gather instruction
SPI = NPI // P             # 8 slots per partition per instruction


def _dma_gather_raw(gpsimd, out_ap, in_ap, idxs_ap, num_idxs, elem_size,
                    elem_step, queue_num):
    """bass.dma_gather without the elem_size%256 restriction (the 256B
    constraint is on the row stride, which we satisfy with PITCH=64 f32)."""
    self = gpsimd
    assert idxs_ap.dtype == mybir.dt.int16
    assert in_ap.space == bass.MemorySpace.DRAM
    assert in_ap.dtype == out_ap.dtype
    assert idxs_ap.space == bass.MemorySpace.SBUF
    assert out_ap.space == bass.MemorySpace.SBUF
    assert ap_utils.ap_is_contiguous(out_ap.ap[1:])
    assert ap_utils.ap_is_contiguous(idxs_ap.ap[1:])
    assert in_ap.ap[-1][1] == out_ap.ap[-1][1] == elem_size
    assert out_ap.ap[0][1] * out_ap.ap[1][1] == round_up_to_multiple(num_idxs, 128)
    assert in_ap.ap[0][0] == elem_step
    stride_bytes = elem_step * mybir.dt.size(in_ap.dtype)
    stride_bytes_256 = exact_div(stride_bytes, 256)
    _in_ap = self.lower_ap_dma(in_ap, for_custom_bir_dma=True)
    _idxs_ap = self.lower_ap(idxs_ap)
    _out_ap = self.lower_ap(out_ap)
    return self.add_instruction(
        mybir.InstDMAGatherAnt(
            name=self.bass.get_next_instruction_name(),
            ins=[*_in_ap, _idxs_ap, self.lower_val_access(self.to_reg(num_idxs))],
            outs=[_out_ap],
            transpose=False,
            num_idxs=num_idxs,
            elem_size=elem_size,
            stride_bytes_256=stride_bytes_256,
            gen_mode=0,
            single_packet=True,
            queue_num=queue_num,
            sbuf_tokens_per_rank=0,
            sbuf_free_dim_per_rank=0,
            sbuf_free_dim_pad_per_rank=0,
            sbuf_byte_offset=0,
        )
    )


def _prep_core(x, w2, idx2, bias_m, rng, K):
    """Build one core's table / idx / weight / bias arrays.

    x: [C, V] f32; w2/idx2: [VPC, R]; bias_m: [VPC]. Returns dict of arrays.
    """
    # color = pos // CSIZE after a fixed permutation of positions
    perm = rng.permutation(V).astype(np.int64)        # table row -> position
    rowof = np.empty(V, np.int64)                     # position -> table row
    rowof[perm] = np.arange(V)

    rows = rowof[idx2]                                # [VPC, R] table rows
    colors = rows // CSIZE                            # [VPC, R]
    local = rows - colors * CSIZE                     # [VPC, R] int16-safe

    # per (voxel, color) slot packing, padded to K (idx 0 / w 0)
    idx16 = np.zeros((VPC, COLORS, K), np.int16)
    wpack = np.zeros((VPC, COLORS, K), np.float32)
    order = np.argsort(colors + np.linspace(0, 0.9, R)[None, :], axis=1, kind="stable")
    csorted = np.take_along_axis(colors, order, axis=1)
    lsorted = np.take_along_axis(local, order, axis=1)
    wsorted = np.take_along_axis(w2, order, axis=1)
    cnt = np.stack([(csorted == c).sum(axis=1) for c in range(COLORS)], axis=1)
    assert cnt.max() <= K, (cnt.max(), K)
    starts = np.concatenate([np.zeros((VPC, 1), np.int64),
                             np.cumsum(cnt, axis=1)[:, :-1]], axis=1)
    # slot index within (voxel, color) for each sorted ref
    pos_in_row = np.arange(R)[None, :] - np.take_along_axis(starts, csorted, axis=1)
    vv = np.repeat(np.arange(VPC), R)
    idx16[vv, csorted.ravel(), pos_in_row.ravel()] = lsorted.ravel().astype(np.int16)
    wpack[vv, csorted.ravel(), pos_in_row.ravel()] = wsorted.ravel() * np.float32(SCALE)

    # table [V, PITCH] f32, rows in perm order, payload = x[:, pos]
    tab = np.zeros((V, PITCH), np.float32)
    tab[:, :CHANNEL] = x[:, perm].T

    # device idx tensor [JPP, 128, COLORS*(K//SPI)*64] int16
    # voxel v = p*JPP + jj ; instruction (c, kk) covers slots kk*SPI..+SPI
    ipc = K // SPI
    t = idx16.reshape(P, JPP, COLORS, ipc, SPI)       # [p, jj, c, kk, q]
    t = t.transpose(1, 2, 3, 4, 0)                    # [jj, c, kk, q, p]
    t = t.reshape(JPP, COLORS * ipc, NPI)             # list i = q*128+p
    t = t.reshape(JPP, COLORS * ipc, NPI // 16, 16)   # wrap 16
    t = np.swapaxes(t, 2, 3)                          # [jj, blk, 16, 64]
    t = np.tile(t, (1, 1, P // 16, 1))                # replicate to 128 parts
    idx_dev = np.ascontiguousarray(
        t.reshape(JPP, COLORS * ipc, P, NPI // 16).transpose(0, 2, 1, 3)
        .reshape(JPP, P, COLORS * ipc * (NPI // 16))
    )

    w_dev = np.ascontiguousarray(
        wpack.reshape(P, JPP, COLORS * K).transpose(1, 0, 2)
    )  # [jj, 128, COLORS*K]

    bias_dev = np.ascontiguousarray(
        np.repeat(bias_m.reshape(P, JPP), CHANNEL).reshape(P, JPP * CHANNEL)
    )  # [128, 512]

    return dict(tab=tab, idx=idx_dev, wts=w_dev, biasx=bias_dev)


def _build_module(K):
    ipc = K // SPI
    nblk = COLORS * ipc
    nc = bacc.Bacc(
        "TRN2",
        target_bir_lowering=False,
        debug=False,
        num_devices=NCORES,
        dynamic_dma_scratch_size=16384,
        num_swdge_queues=4,
    )
    tab_d = nc.dram_tensor("tab", [V, PITCH], mybir.dt.float32, kind="ExternalInput")
    idx_d = nc.dram_tensor("idx", [JPP, P, nblk * (NPI // 16)], mybir.dt.int16,
                           kind="ExternalInput")
    w_d = nc.dram_tensor("wts", [JPP, P, COLORS * K], mybir.dt.float32,
                         kind="ExternalInput")
    b_d = nc.dram_tensor("biasx", [P, JPP * CHANNEL], mybir.dt.float32,
                         kind="ExternalInput")
    out_d = nc.dram_tensor("out", [P, JPP * CHANNEL], mybir.dt.float32,
                           kind="ExternalOutput")

    idx_ap = idx_d.ap()
    w_ap = w_d.ap()
    tab_ap = tab_d.ap()

    with tile.TileContext(nc) as tc:
        with (
            tc.tile_pool(name="const", bufs=1) as cp,
            tc.tile_pool(name="stream", bufs=2) as sp,
            tc.tile_pool(name="gat", bufs=2) as gp,
        ):
            bias_t = cp.tile([P, JPP * CHANNEL], mybir.dt.float32)
            out_t = cp.tile([P, JPP * CHANNEL], mybir.dt.float32)
            nc.sync.dma_start(out=bias_t[:], in_=b_d.ap())

            for jj in range(JPP):
                idx_t = sp.tile([P, nblk * (NPI // 16)], mybir.dt.int16, tag="idx")
                w_t = sp.tile([P, COLORS * K], mybir.dt.float32, tag="w")
                g_t = gp.tile([P, COLORS * K * CHANNEL], mybir.dt.float32, tag="g")
                nc.sync.dma_start(out=idx_t[:], in_=idx_ap[jj])
                nc.sync.dma_start(out=w_t[:], in_=w_ap[jj])
                for c in range(COLORS):
                    src = tab_ap[c * CSIZE:(c + 1) * CSIZE, :CHANNEL]
                    for kk in range(ipc):
                        blk = c * ipc + kk
                        slot0 = (c * K + kk * SPI) * CHANNEL
                        out_ap = g_t[:, slot0:slot0 + SPI * CHANNEL].rearrange(
                            "p (q e) -> p q e", e=CHANNEL)
                        _dma_gather_raw(
                            nc.gpsimd,
                            out_ap=out_ap,
                            in_ap=src,
                            idxs_ap=idx_t[:, blk * (NPI // 16):(blk + 1) * (NPI // 16)],
                            num_idxs=NPI,
                            elem_size=CHANNEL,
                            elem_step=PITCH,
                            queue_num=blk % 4,
                        )
                g3 = g_t[:].rearrange("p (k e) -> p k e", e=CHANNEL)
                wb = w_t[:].to_broadcast([P, COLORS * K, CHANNEL])
                nc.vector.tensor_tensor(out=g3, in0=g3, in1=wb,
                                        op=mybir.AluOpType.mult)
                gr = g_t[:].rearrange("p (k e) -> p e k", e=CHANNEL)
                o3 = out_t[:, jj * CHANNEL:(jj + 1) * CHANNEL].rearrange(
                    "p (one e) -> p one e", one=1)
                nc.vector.tensor_reduce(out=o3, in_=gr,
                                        axis=mybir.AxisListType.X,
                                        op=mybir.AluOpType.add)
            nc.vector.tensor_tensor(out=out_t[:], in0=out_t[:], in1=bias_t[:],
                                    op=mybir.AluOpType.add)
            nc.sync.dma_start(out=out_d.ap(), in_=out_t[:])

    nc.compile()
    nc.m = get_hw_module(nc.m)
    return nc


class _Runner:
    """Compile once, execute the SPMD module on 8 cores via PJRT."""

    def __init__(self, nc, n_cores):
        import jax
        from jax.sharding import Mesh, PartitionSpec
        from jax.experimental.shard_map import shard_map
        from concourse.bass2jax import (_bass_exec_p, partition_id_tensor,
                                        install_neuronx_cc_hook)

        install_neuronx_cc_hook()
        self.jax = jax
        self.n_cores = n_cores
        in_names, out_names, out_avals = [], [], []
        pname = nc.partition_id_tensor.name if nc.partition_id_tensor else None
        for alloc in nc.m.functions[0].allocations:
            if not isinstance(alloc, mybir.MemoryLocationSet):
                continue
            name = alloc.memorylocations[0].name
            if alloc.kind == "ExternalInput":
                if name != pname:
                    in_names.append(name)
            elif alloc.kind == "ExternalOutput":
                out_names.append(name)
                out_avals.append(jax.core.ShapedArray(
                    tuple(alloc.tensor_shape), mybir.dt.np(alloc.dtype)))
        self.in_names, self.out_names, self.out_avals = in_names, out_names, out_avals
        all_in = list(in_names) + list(out_names) + ([pname] if pname else [])

        def _body(*args):
            operands = list(args)
            if pname is not None:
                operands.append(partition_id_tensor())
            return tuple(_bass_exec_p.bind(
                *operands, out_avals=tuple(out_avals), in_names=tuple(all_in),
                out_names=tuple(out_names), lowering_input_output_aliases=(),
                sim_require_finite=True, sim_require_nnan=True, nc=nc))

        devices = jax.devices()[:n_cores]
        self.mesh = Mesh(np.asarray(devices), ("core",))
        nin = len(in_names) + len(out_names)
        self.fn = jax.jit(
            shard_map(_body, mesh=self.mesh,
                      in_specs=(PartitionSpec("core"),) * nin,
                      out_specs=(PartitionSpec("core"),) * len(out_names),
                      check_rep=False),
            keep_unused=True)
        self._dev_in = None

    def set_inputs(self, in_maps):
        import jax
        from jax.sharding import NamedSharding, PartitionSpec
        sh = NamedSharding(self.mesh, PartitionSpec("core"))
        n = self.n_cores
        cat = [np.concatenate([np.asarray(in_maps[c][nm]) for c in range(n)], axis=0)
               for nm in self.in_names]
        zeros = [np.zeros((n * a.shape[0], *a.shape[1:]), a.dtype)
                 for a in self.out_avals]
        self._dev_in = [jax.device_put(x, sh) for x in cat + zeros]

    def run(self):
        outs = self.fn(*self._dev_in)
        self.jax.block_until_ready(outs)
        return outs

    def outputs_np(self, outs):
        n = self.n_cores
        return [
            {nm: np.asarray(outs[i]).reshape(n, *self.out_avals[i].shape)[c]
             for i, nm in enumerate(self.out_names)}
            for c in range(n)
        ]


_CACHE = {}


def _get_runner(K):
    if K not in _CACHE:
        nc = _build_module(K)
        _CACHE[K] = _Runner(nc, NCORES)
    return _CACHE[K]


def prepare(x, weight, bias, indices):
    """Host-side marshalling: shard + build per-core device arrays."""
    x = np.asarray(x, np.float32).reshape(CHANNEL, V)
    weight = np.asarray(weight, np.float32).reshape(NVOX, R)
    bias = np.asarray(bias, np.float32).reshape(NVOX)
    indices = np.asarray(indices).astype(np.int64).reshape(NVOX, R)

    rngs = [np.random.default_rng(1234 + m) for m in range(NCORES)]
    # K must be uniform (SPMD): probe counts first with each core's coloring
    Ks = []
    percore = []
    for m in range(NCORES):
        sl = slice(m * VPC, (m + 1) * VPC)
        percore.append((indices[sl], weight[sl], bias[sl]))
    # quick max-count probe
    maxc = 0
    perms = []
    for m in range(NCORES):
        perm = rngs[m].permutation(V).astype(np.int64)
        perms.append(perm)
        rowof = np.empty(V, np.int64)
        rowof[perm] = np.arange(V)
        colors = rowof[percore[m][0]] // CSIZE
        cnt = np.stack([(colors == c).sum(axis=1) for c in range(COLORS)], axis=1)
        maxc = max(maxc, int(cnt.max()))
    K = int(round_up_to_multiple(max(maxc, 128), SPI))

    in_maps = []
    for m in range(NCORES):
        idx2, w2, bias_m = percore[m]
        rng = np.random.default_rng(1234 + m)  # same perm as probe
        in_maps.append(_prep_core(x, w2, idx2, bias_m, rng, K))
    return K, in_maps


def kernel(x, weight, bias, indices):
    K, in_maps = prepare(x, weight, bias, indices)
    runner = _get_runner(K)
    runner.set_inputs(in_maps)
    outs = runner.run()
    per_core = runner.outputs_np(outs)
    full = np.empty((1, CHANNEL, NVOX), np.float32)
    for m in range(NCORES):
        o = per_core[m]["out"].reshape(P, JPP, CHANNEL)
        full[0, :, m * VPC:(m + 1) * VPC] = o.transpose(2, 0, 1).reshape(CHANNEL, VPC)
    return full.reshape(1, CHANNEL, NVX, NVY)

